# revision 4
# baseline (speedup 1.0000x reference)
"""Trainium2 Bass kernel for causal dynamic (MoE-routed) attention.

Problem: y = (softmax-routed top-4-of-16-heads causal attention)(x) @ W_o
  x [B=2, T=2048, D=1024], W_qkv [D, 3D], W_router [D, 16], W_o [D, D].

Sharding (8 cores): core c -> batch b = c // 4, head-group hg = c % 4
(4 of 16 heads). Each core computes a partial y contribution of its 4
heads for its batch; host sums the 4 partials per batch (row-parallel
W_o unshard) and stacks batches.

Design:
  - S^T = K @ Q^T per [128k x 256q] block on PE (bf16 Q/K), exp
    evacuates PSUM -> SBUF as bf16 (softmax scale folded in), causal
    masks (DVE, bf16 2x mode) touch only the diagonal group, whose
    fully-masked half-block is skipped outright (exact-causal S).
  - PV is transposed: out[q, dh] with queries on PSUM partitions and
    ap=65 (dh + a ones-column that yields the softmax denominator),
    bf16 operands.  This halves PV tensor-engine time vs a dims-major
    layout and puts the denominator/gate per-partition, so gating is
    three tiny DVE ops per (head, 128q) with no broadcast round trip.
    All four (head, q-half) accumulators share one PSUM bank; only the
    first matmul carries start=True because the start flag marks the
    whole 2KB zero-region pending-zero.
  - Y (token-major bf16) is flipped to head-dim-major for W_o with PE
    transposes (DMA-transpose completion sems race PE consumers on HW).
  - Single flat emission: V+router projection, QK projection, both
    head-halves of attention, W_o and output DMA interleave through a
    deferred-work queue drained at attention-group boundaries, so no
    engine starves; W_o quads are held back for the long final pairs
    where ACT (exp) outpaces PE.  Each pair's last group is finished
    inside the next pair (software pipelining across boundaries), and
    the diagonal group is processed first so its mask latency hides.
  - The router runs in 4-block quarters right behind the V projection
    so early pairs can gate; inputs arrive as a few large DMAs (each
    HWDGE slot is ~630ns) ordered to unblock the first matmuls ~5us in.
"""

import os
import sys

import numpy as np

for _p in ("/opt/trn_rl_repo", "/root/.axon_site/_ro/trn_rl_repo"):
    if os.path.isdir(_p) and _p not in sys.path:
        sys.path.insert(0, _p)

import concourse.bacc as bacc
import concourse.bass as bass
import concourse.mybir as mybir
import concourse.tile as tile
from concourse.bass_utils import run_bass_kernel_spmd

F32 = mybir.dt.float32
F32R = mybir.dt.float32r
BF16 = mybir.dt.bfloat16
AF = mybir.ActivationFunctionType
ALU = mybir.AluOpType
AX = mybir.AxisListType

B = 2
D = 1024
H_TOTAL = 16
H_ACTIVE = 4
DH = 64          # head dim
HPC = 4          # heads per core
N_CORES = 8
NEG_BIG = -1.0e30


def _bcast_inner(ap, n):
    """View an AP as [..., n] with step-0 innermost broadcast."""
    return bass.AP(
        tensor=ap.tensor,
        offset=ap.offset,
        ap=[*ap.ap, [0, n]],
    )


def _bcast_mid(ap2d, n):
    """View a [P, C] AP as [P, n, C] with a step-0 middle broadcast."""
    return bass.AP(
        tensor=ap2d.tensor,
        offset=ap2d.offset,
        ap=[ap2d.ap[0], [0, n], ap2d.ap[-1]],
    )


def build_nc(T):
    """Build the single-core Bass module (SPMD across 8 cores via inputs)."""
    QB = T // 128    # 128-token query blocks (16)
    QP = T // 256    # 256-token query pairs (8)
    DC = D // 128    # contraction chunks (8)
    NQK = T // 512   # 512-token chunks (4)

    nc = bacc.Bacc("TRN2", target_bir_lowering=False, debug=False)

    xT = nc.dram_tensor("xT", [D, T], F32R, kind="ExternalInput")
    wqk = nc.dram_tensor("wqk", [D, 512], F32R, kind="ExternalInput")
    wvr = nc.dram_tensor("wvr", [D, 272], F32R, kind="ExternalInput")
    wo = nc.dram_tensor("wo", [256, D], BF16, kind="ExternalInput")
    tri = nc.dram_tensor("tri", [128, 512], BF16, kind="ExternalInput")
    iden = nc.dram_tensor("iden", [128, 128], BF16, kind="ExternalInput")
    out = nc.dram_tensor("out", [T, D], F32, kind="ExternalOutput")

    with tile.TileContext(nc) as tc:
        with (
            tc.tile_pool(name="persist", bufs=1) as persist,
            tc.tile_pool(name="router", bufs=2) as rpool,
            tc.tile_pool(name="ppsum", bufs=2, space="PSUM") as ppsum,
            tc.tile_pool(name="stpsum", bufs=2, space="PSUM") as stpsum,
            tc.tile_pool(name="accpsum", bufs=2, space="PSUM") as accpsum,
            tc.tile_pool(name="ptpool", bufs=4) as ptpool,
            tc.tile_pool(name="yspool", bufs=16) as yspool,
            tc.tile_pool(name="ostage", bufs=3) as ostage,
            tc.tile_pool(name="smalls", bufs=16) as smalls,
        ):
            # ---- persistent SBUF tensors ----
            xb = persist.tile([128, DC, T], F32R, tag="xb", name="xb")
            wqkb = persist.tile([128, DC, 512], F32R, tag="wqkb", name="wqkb")
            wvrb = persist.tile([128, DC, 272], F32R, tag="wvrb", name="wvrb")
            wo_sb = persist.tile([128, 2, D], BF16, tag="wo", name="wo_sb")
            QT = [persist.tile([128, T], BF16, tag=f"QT{t}", name=f"QT{t}")
                  for t in range(2)]
            KT = [persist.tile([128, T], BF16, tag=f"KT{t}", name=f"KT{t}")
                  for t in range(2)]
            Vt = persist.tile([128, QB * HPC * 65], BF16, tag="Vt", name="Vt")
            Vt4 = Vt.rearrange("p (q h e) -> p q h e", q=QB, h=HPC)
            YT = persist.tile([128, 2, T], BF16, tag="YT", name="YT")
            tri_sb = persist.tile([128, 512], BF16, tag="tri", name="tri_sb")
            iden_sb = persist.tile([128, 128], BF16, tag="iden", name="iden_sb")
            RTlog = persist.tile([128, QB * 16], F32, tag="RTlog", name="RTlog")
            G = persist.tile([128, QB * 16], F32, tag="G", name="G")

            # ---- input DMAs, ordered to unblock compute ASAP ----
            xT3 = xT.rearrange("(d p) t -> p d t", p=128)
            wqk3 = wqk.rearrange("(d p) c -> p d c", p=128)
            wvr3 = wvr.rearrange("(d p) c -> p d c", p=128)
            nc.sync.dma_start(out=wvrb[:, 0:4, :], in_=wvr3[:, 0:4, :])
            nc.sync.dma_start(out=xb[:, 0:4, 0:128], in_=xT3[:, 0:4, 0:128])
            nc.sync.dma_start(out=wvrb[:, 4:8, :], in_=wvr3[:, 4:8, :])
            nc.sync.dma_start(out=xb[:, 4:8, 0:128], in_=xT3[:, 4:8, 0:128])
            nc.sync.dma_start(out=xb[:, :, 128:256], in_=xT3[:, :, 128:256])
            nc.sync.dma_start(out=xb[:, :, 256:512], in_=xT3[:, :, 256:512])
            for m in (0, 2, 1, 3):
                msl = slice(128 * m, 128 * m + 128)
                nc.sync.dma_start(out=wqkb[:, :, msl], in_=wqk3[:, :, msl])
            nc.sync.dma_start(out=xb[:, :, 512:768], in_=xT3[:, :, 512:768])
            nc.sync.dma_start(out=tri_sb, in_=tri[:, :])
            nc.sync.dma_start(out=iden_sb, in_=iden[:, :])
            nc.sync.dma_start(out=xb[:, :, 768:1024], in_=xT3[:, :, 768:1024])
            nc.sync.dma_start(out=xb[:, :, 1024:1536], in_=xT3[:, :, 1024:1536])
            nc.sync.dma_start(out=wo_sb, in_=wo.rearrange("(k p) d -> p k d", p=128))
            nc.sync.dma_start(out=xb[:, :, 1536:2048], in_=xT3[:, :, 1536:2048])
            # ones column of the augmented V (the softmax denominator row)
            nc.vector.memset(Vt4[:, :, :, 64:65], 1.0)

            # ================= emission helpers =================

            def emit_vproj(q):
                ps = ppsum.tile([128, 512], F32, tag="ps", name="vps")
                for d in range(DC):
                    nc.tensor.matmul(
                        out=ps[:, 0:272],
                        lhsT=xb[:, d, 128 * q:128 * q + 128],
                        rhs=wvrb[:, d, :],
                        start=(d == 0),
                        stop=(d == DC - 1),
                    )
                nc.vector.tensor_copy(
                    out=Vt4[:, q, :, 0:64],
                    in_=ps[:, 0:256].rearrange("p (h e) -> p h e", h=HPC),
                )
                nc.vector.tensor_copy(out=RTlog[:, 16 * q:16 * q + 16],
                                      in_=ps[:, 256:272])

            dests = {0: QT[0], 1: QT[1], 2: KT[0], 3: KT[1]}

            def emit_qkproj(n, m):
                ns = slice(512 * n, 512 * n + 512)
                ps = ppsum.tile([128, 512], F32, tag="ps", name="qkps")
                for d in range(DC):
                    nc.tensor.matmul(
                        out=ps,
                        lhsT=wqkb[:, d, 128 * m:128 * m + 128],
                        rhs=xb[:, d, ns],
                        start=(d == 0),
                        stop=(d == DC - 1),
                    )
                nc.vector.tensor_copy(out=dests[m][:, ns], in_=ps)

            def emit_router_quarter(h):
                """softmax + top-4 gates for q-blocks [4h, 4h+4)."""
                csl = slice(64 * h, 64 * h + 64)
                E = rpool.tile([128, 64], F32, tag="E", name="E")
                W = rpool.tile([128, 64], F32, tag="W", name="W")
                M = rpool.tile([128, 4], F32, tag="M", name="M")
                SS = rpool.tile([128, 4], F32, tag="SS", name="SS")
                ZR = rpool.tile([128, 4], F32, tag="ZR", name="ZR")
                # e = exp(logits); logits are ~N(0,1) so no max-subtraction
                nc.scalar.activation(out=E, in_=RTlog[:, csl], func=AF.Exp)
                E3 = E.rearrange("p (q h) -> p q h", h=16)
                W3 = W.rearrange("p (q h) -> p q h", h=16)
                nc.vector.tensor_reduce(out=SS, in_=E3, axis=AX.X, op=ALU.add)
                nc.vector.reciprocal(out=ZR, in_=SS)
                nc.vector.tensor_copy(out=W, in_=E)
                # peel off the 3 largest per (token, 16-head group)
                for _ in range(3):
                    nc.vector.tensor_reduce(out=M, in_=W3, axis=AX.X, op=ALU.max)
                    C = rpool.tile([128, 64], F32, tag="C", name="C")
                    nc.vector.tensor_tensor(
                        out=C.rearrange("p (q h) -> p q h", h=16),
                        in0=W3,
                        in1=_bcast_inner(M, 16),
                        op=ALU.is_ge,
                    )
                    nc.vector.scalar_tensor_tensor(
                        out=W, in0=C, scalar=NEG_BIG, in1=W,
                        op0=ALU.mult, op1=ALU.add,
                    )
                # m4 = 4th largest; gates = e * (e >= m4) / sum
                nc.vector.tensor_reduce(out=M, in_=W3, axis=AX.X, op=ALU.max)
                C4 = rpool.tile([128, 64], F32, tag="C", name="C4")
                nc.vector.tensor_tensor(
                    out=C4.rearrange("p (q h) -> p q h", h=16),
                    in0=E3,
                    in1=_bcast_inner(M, 16),
                    op=ALU.is_ge,
                )
                Gh = G[:, csl]
                nc.vector.tensor_tensor(out=Gh, in0=E, in1=C4, op=ALU.mult)
                nc.vector.tensor_tensor(
                    out=Gh.rearrange("p (q h) -> p q h", h=16),
                    in0=Gh.rearrange("p (q h) -> p q h", h=16),
                    in1=_bcast_inner(ZR, 16),
                    op=ALU.mult,
                )

            # Y staging tiles by 128-token block index b (live across both
            # head-half passes of a pair, until the transposes read them)
            ys = {}

            # deferred PE-side work, drained one item per attention-group
            # boundary so ACT/DVE never starve while PE runs projections.
            # W_o quads live in their own queue so some can be held back
            # for the long final pairs, where ACT (exp) outpaces PE.
            work_q = []
            wo_q = []
            wo_reserve = [0]

            def filler():
                if work_q:
                    work_q.pop(0)[1]()
                elif len(wo_q) > wo_reserve[0]:
                    wo_q.pop(0)()

            def drain(tag):
                if tag == "wo":
                    while wo_q:
                        wo_q.pop(0)()
                    return
                rest = []
                for it in work_q:
                    if it[0] == tag:
                        it[1]()
                    else:
                        rest.append(it)
                work_q[:] = rest

            def emit_wo_tp(b):
                qs = slice(128 * b, 128 * b + 128)
                # flip Y to head-dim-major via PE transpose (DMA-transpose
                # completion sems race PE consumers on hardware)
                for k in range(2):
                    tp = ppsum.tile([128, 128], BF16, tag="ps", name="tp")
                    nc.tensor.transpose(
                        out=tp,
                        in_=ys[b][:, 128 * k:128 * k + 128],
                        identity=iden_sb,
                    )
                    nc.vector.tensor_copy(out=YT[:, k, qs], in_=tp)

            def emit_wo_b(b):
                qs = slice(128 * b, 128 * b + 128)
                stage = ostage.tile([128, D], F32, tag="stage", name="stage")
                for nh in range(2):
                    nsl = slice(512 * nh, 512 * nh + 512)
                    ps = ppsum.tile([128, 512], F32, tag="ps", name="wops")
                    for k in range(2):
                        nc.tensor.matmul(
                            out=ps,
                            lhsT=YT[:, k, qs],
                            rhs=wo_sb[:, k, nsl],
                            start=(k == 0), stop=(k == 1),
                        )
                    if b >= 14 and nh == 1:
                        # tail: split the two stage halves across ACT+DVE
                        nc.scalar.copy(out=stage[:, nsl], in_=ps)
                    else:
                        nc.vector.tensor_copy(out=stage[:, nsl], in_=ps)
                    if b >= 12:
                        # tail: start each half's output DMA immediately
                        nc.sync.dma_start(out=out[qs, nsl],
                                          in_=stage[:, nsl])
                if b < 12:
                    nc.sync.dma_start(out=out[qs, :], in_=stage)

            # continuation that finishes the previous pair's last group
            pending = [None]

            def emit_pair(t, p):
                qsl = slice(256 * p, 256 * p + 256)
                njs = 2 * p + 2          # causal key blocks for this pair
                ngrp = p + 1             # j-groups of 2 blocks
                accQ = accpsum.tile([128, 260], F32, tag="acc", name="accQ")
                sts = [None] * ngrp
                first_pv = [True]
                # diagonal group first: its mask latency hides behind the
                # bulk groups instead of stalling the pair's tail
                order = [ngrp - 1] + list(range(ngrp - 1))

                qsl_hi = slice(256 * p + 128, 256 * p + 256)

                def s_group(g):
                    st = stpsum.tile([128, 1024], F32, tag="st", name="st")
                    diag = (g == ngrp - 1)
                    # diagonal group layout [A2p A2p+1h pad | B2p B2p+1h pad]
                    # (the low query half of block 2p+1 is fully masked; the
                    # pad keeps each head-half inside one PSUM bank)
                    w1 = 128 if diag else 256
                    for hl, tp_ in ((0, (0, 0)), (1, (64, 0))):
                        ksl = slice(64 * hl, 64 * hl + 64)
                        j = 2 * g
                        nc.tensor.matmul(
                            out=st[:, 512 * hl:512 * hl + 256],
                            lhsT=KT[t][ksl, 128 * j:128 * j + 128],
                            rhs=QT[t][ksl, qsl],
                            start=True, stop=True,
                            tile_position=tp_,
                        )
                        j = 2 * g + 1
                        nc.tensor.matmul(
                            out=st[:, 512 * hl + 256:512 * hl + 256 + w1],
                            lhsT=KT[t][ksl, 128 * j:128 * j + 128],
                            rhs=QT[t][ksl, qsl_hi if diag else qsl],
                            start=True, stop=True,
                            tile_position=tp_,
                        )
                    sts[g] = st

                pts = [None] * ngrp

                def exp_group(g):
                    diag = (g == ngrp - 1)
                    pt = ptpool.tile([128, 1024], BF16, tag="pt", name="pt")
                    if diag:
                        # strided exp skips the pad columns
                        nc.scalar.activation(
                            out=bass.AP(tensor=pt.tensor, offset=pt.offset,
                                        ap=[pt.ap[0], [512, 2], [1, 384]]),
                            in_=bass.AP(tensor=sts[g].tensor,
                                        offset=sts[g].offset,
                                        ap=[sts[g].ap[0], [512, 2], [1, 384]]),
                            func=AF.Exp, scale=0.125)
                        # causal masks: triu|ones on block 2p, triu on the
                        # high-query half of block 2p+1 (both heads at once)
                        nc.vector.tensor_tensor(
                            out=bass.AP(tensor=pt.tensor, offset=pt.offset,
                                        ap=[pt.ap[0], [512, 2], [1, 256]]),
                            in0=bass.AP(tensor=pt.tensor, offset=pt.offset,
                                        ap=[pt.ap[0], [512, 2], [1, 256]]),
                            in1=_bcast_mid(tri_sb[:, 0:256], 2),
                            op=ALU.mult,
                        )
                        nc.vector.tensor_tensor(
                            out=bass.AP(tensor=pt.tensor, offset=pt.offset + 256,
                                        ap=[pt.ap[0], [512, 2], [1, 128]]),
                            in0=bass.AP(tensor=pt.tensor, offset=pt.offset + 256,
                                        ap=[pt.ap[0], [512, 2], [1, 128]]),
                            in1=_bcast_mid(tri_sb[:, 0:128], 2),
                            op=ALU.mult,
                        )
                    else:
                        nc.scalar.activation(out=pt, in_=sts[g],
                                             func=AF.Exp, scale=0.125)
                    pts[g] = pt

                def finish_group(g):
                    diag = (g == ngrp - 1)
                    pt = pts[g]
                    # start=True zeroes the whole 2KB PSUM zero-region
                    # (bank) lazily, so only the very first matmul into the
                    # accQ bank may carry it; the other regions' first
                    # writes are zeroed by the same pending-zero mark.
                    for jj in range(2):
                        j = 2 * g + jj
                        for hl in range(2):
                            for h0 in range(2):
                                if diag and jj == 1 and h0 == 0:
                                    continue  # fully masked half-block
                                r = 2 * hl + h0
                                if diag and jj == 1:
                                    lo = 512 * hl + 256
                                else:
                                    lo = 512 * hl + 256 * jj + 128 * h0
                                nc.tensor.matmul(
                                    out=accQ[:, 65 * r:65 * r + 65],
                                    lhsT=pt[:, lo:lo + 128],
                                    rhs=Vt4[:, j, 2 * t + hl, :],
                                    start=(first_pv[0] and jj == 0 and r == 0),
                                    stop=(g == order[-1] and jj == 1),
                                    skip_group_check=True,
                                )
                    first_pv[0] = False

                def finish_pair():
                    finish_group(order[-1])
                    # gating: y = P V * gate / denom, all per-partition now
                    for hl in range(2):
                        head = 2 * t + hl
                        for h0 in range(2):
                            r = 2 * hl + h0
                            b = 2 * p + h0
                            if t == 0 and hl == 0:
                                ys[b] = yspool.tile([128, 256], BF16,
                                                    tag="ys", name="ysb")
                            rcp = smalls.tile([128, 1], F32, tag="rcp",
                                              name="rcp")
                            nc.vector.reciprocal(
                                out=rcp, in_=accQ[:, 65 * r + 64:65 * r + 65])
                            w = smalls.tile([128, 1], F32, tag="w", name="w")
                            nc.vector.tensor_tensor(
                                out=w, in0=rcp,
                                in1=G[:, 16 * b + head:16 * b + head + 1],
                                op=ALU.mult,
                            )
                            nc.vector.tensor_tensor(
                                out=ys[b][:, 64 * head:64 * head + 64],
                                in0=accQ[:, 65 * r:65 * r + 64],
                                in1=_bcast_inner(w[:, 0], 64),
                                op=ALU.mult,
                            )
                    if t == 1:
                        for h0 in range(2):
                            b = 2 * p + h0
                            wo_q.append(lambda b=b: emit_wo_tp(b))
                            wo_q.append(lambda b=b: emit_wo_b(b))

                s_group(order[0])
                exp_group(order[0])
                if pending[0] is not None:
                    pending[0]()
                    pending[0] = None
                for i in range(1, ngrp):
                    s_group(order[i])
                    exp_group(order[i])
                    filler()
                    finish_group(order[i - 1])
                pending[0] = finish_pair

            # ================= main emission =================
            emit_vproj(0)
            emit_vproj(1)
            emit_vproj(2)
            emit_vproj(3)
            emit_qkproj(0, 0)
            emit_qkproj(0, 2)
            emit_router_quarter(0)
            emit_pair(0, 0)
            emit_qkproj(0, 1)
            emit_qkproj(0, 3)
            emit_pair(1, 0)

            def push_chunk_work(n1):
                for q in range(4 * n1, 4 * n1 + 4):
                    work_q.append(("vp", lambda q=q: emit_vproj(q)))
                for m in range(4):
                    work_q.append(("qk", lambda m=m, n1=n1: emit_qkproj(n1, m)))

            for n in range(NQK):
                wo_reserve[0] = 28 if n < NQK - 1 else 4
                if n >= 1:
                    drain("vp")
                    emit_router_quarter(n)
                    drain("qk")
                if n == 2:
                    push_chunk_work(3)
                for idx, p in enumerate((2 * n, 2 * n + 1)):
                    for t in (0, 1):
                        if n == 0 and p == 0:
                            continue
                        emit_pair(t, p)
                    if idx == 0 and n < 2:
                        push_chunk_work(n + 1)

            pending[0]()
            pending[0] = None
            drain("wo")

    nc.compile()
    return nc


_NC_CACHE = {}


def _get_nc(T):
    if T not in _NC_CACHE:
        _NC_CACHE[T] = build_nc(T)
    return _NC_CACHE[T]


def make_in_maps(x, W_qkv, W_router, W_o):
    """Shard full inputs into the 8 per-core input maps."""
    import ml_dtypes

    x = np.asarray(x, dtype=np.float32)
    W_qkv = np.asarray(W_qkv, dtype=np.float32)
    W_router = np.asarray(W_router, dtype=np.float32)
    W_o = np.asarray(W_o, dtype=np.float32)
    Bx, T, Dx = x.shape
    T_ = np.triu(np.ones((128, 128), dtype=np.float32))  # T_[k, q] = q >= k
    tri1 = np.concatenate([T_, np.ones((128, 128), np.float32)], axis=1)
    tri2 = np.concatenate([np.zeros((128, 128), np.float32), T_], axis=1)
    tri = np.concatenate([tri1, tri2], axis=1).astype(ml_dtypes.bfloat16)
    in_maps = []
    for c in range(N_CORES):
        b, hg = c // 4, c % 4
        csl = slice(256 * hg, 256 * hg + 256)
        xTc = np.ascontiguousarray(x[b].T)
        wqk = np.ascontiguousarray(
            np.concatenate([W_qkv[:, csl],
                            W_qkv[:, 1024 + 256 * hg:1024 + 256 * hg + 256]],
                           axis=1)
        )
        perm = (list(range(4 * hg, 4 * hg + 4))
                + [h for h in range(16) if not (4 * hg <= h < 4 * hg + 4)])
        wvr = np.ascontiguousarray(
            np.concatenate([W_qkv[:, 2048 + 256 * hg:2048 + 256 * hg + 256],
                            W_router[:, perm]], axis=1)
        )
        wo = np.ascontiguousarray(W_o[csl, :]).astype(ml_dtypes.bfloat16)
        iden = np.eye(128, dtype=np.float32).astype(ml_dtypes.bfloat16)
        in_maps.append({
            "xT": xTc, "wqk": wqk, "wvr": wvr, "wo": wo, "tri": tri,
            "iden": iden,
        })
    return in_maps


def kernel_raw(x, W_qkv, W_router, W_o, **run_kwargs):
    """Run on the 8 cores; returns (full_output, BassKernelResults)."""
    import time

    T = x.shape[1]
    nc = _get_nc(T)
    in_maps = make_in_maps(x, W_qkv, W_router, W_o)
    last_exc = None
    for attempt in range(3):
        try:
            res = run_bass_kernel_spmd(nc, in_maps,
                                       core_ids=list(range(N_CORES)),
                                       **run_kwargs)
            break
        except Exception as e:  # transient NRT_EXEC_UNIT_UNRECOVERABLE etc.
            last_exc = e
            if attempt == 2:
                raise
            time.sleep(20)
    partials = [r["out"] for r in res.results]
    y = np.stack([
        partials[0] + partials[1] + partials[2] + partials[3],
        partials[4] + partials[5] + partials[6] + partials[7],
    ]).astype(np.float32)
    return y, res


def kernel(x, W_qkv, W_router, W_o):
    y, _ = kernel_raw(x, W_qkv, W_router, W_o)
    return y
